# revision 1
# baseline (speedup 1.0000x reference)
"""DETR self-attention Bass/Trainium2 kernel.

Problem: nn_DetrAttention (B=8, T=2048, E=256, H=8, Dh=32), 8 NeuronCores.
Sharding: data-parallel over batch -- one batch element per core.

Per-core dataflow (all matmuls contract along the SBUF partition dim):
  - host passes hidden[b].T and object_queries[b].T as [E, T] f32, and the
    q/k/v weights as W.T [E, E] bf16, so no on-chip transposes are needed.
  - hs_posT = hiddenT + objT (DVE, f32 -> bf16)
  - qT/kT = W.T-stationary matmuls -> [E, T] bf16 (bias fused in evacuation)
  - v'    = hiddenT-stationary matmul -> natural [T, Dh] layout per head with
    a ones column appended (vprime[:, s, h, 0:32]=v, [...,32]=1)
  - scoresT[s,t] = sum_d kT[d,s] qT[d,t]: row-tiled (32-row) matmul pairs,
    2 heads per PSUM scores tile [128, 2x512]
  - exp on ScalarE straight out of PSUM (scale=1/sqrt(Dh) folded into the
    activation), bf16 out -> this is the kernel's throughput floor
  - attn numerator+denominator in one chain: num'[0:33,t] = v'.T @ expT
    accumulated over s in a dedicated PSUM bank per head (sequential
    accumulation groups -- PSUM start=True zeroes a whole bank region)
  - normalize: reciprocal(num'[32]) -> DMA-bounce to partition 0 -> K=1
    ones-matmul broadcast to partitions 0..31 -> DVE multiply -> attn piece
    [32, TS] bf16 at partitions 0-31
  - output proj: Wo passed head-sliced as wo8[32, h, e_out]; accumulate the
    8 per-head (K=32) matmuls into one PSUM bank, add bias, DMA out as
    out.T [E, T] f32; host re-transposes.

attention_mask is additive and all-zeros by the problem spec (fill: zeros);
the kernel skips it on HW. A host-side guard falls back to an exact numpy
path in the (never-occurring) case of a nonzero mask.
"""

import numpy as np
import ml_dtypes

import concourse.bass as bass
import concourse.mybir as mybir
import concourse.tile as tile
from concourse.bass import ts, ds
from concourse import bass_utils

F32 = mybir.dt.float32
BF16 = mybir.dt.bfloat16
AF = mybir.ActivationFunctionType

B = 8
E = 256
H = 8
DH = 32
P = 128
SCALING = DH ** -0.5
NCORES = 8


def build_nc(T=2048, reps=1):
    """Build the single-core Bass program (same program runs SPMD on 8 cores).

    reps>1 repeats the whole computation (for wall-clock differencing in
    test harnesses); the grading entry point always uses reps=1.
    """
    TS = min(512, T)          # t-block (columns of scores processed at once)
    nc = bass.Bass("TRN2", debug=False, num_devices=NCORES)

    def din(name, shape, dt):
        return nc.dram_tensor(name, shape, dt, kind="ExternalInput").ap()

    hsT = din("hsT", [E, T], F32)
    oqT = din("oqT", [E, T], F32)
    wq = din("wq", [E, E], BF16)        # Wq.T  (lhsT layout: [e_in, e_out])
    wk = din("wk", [E, E], BF16)
    wv = din("wv", [E, E], BF16)
    wo8 = din("wo8", [DH, H * E], BF16)  # Wo.T head-sliced: [d, h*E + e_out]
    bq = din("bq", [E, 1], F32)
    bk = din("bk", [E, 1], F32)
    bo = din("bo", [E, 1], F32)
    outT = nc.dram_tensor("outT", [E, T], F32, kind="ExternalOutput").ap()

    hoist_sem = nc.alloc_semaphore("hoistw")
    with tile.TileContext(nc) as tc:
        for _ in range(reps):
            _body(tc, T, TS, outT, hsT, oqT, wq, wk, wv, wo8, bq, bk, bo)
    # populate .instr bytes for extended gpsimd InstISA (partition_broadcast);
    # Bacc.compile does this but the raw Bass/Tile path does not.
    mybir.codegen_inst_isa_subclasses(nc)
    _drop_own_engine_waits(nc, hoist_sem)
    return nc


def _sem_id(nc, sem):
    return nc.sem_num(sem) if hasattr(nc, "sem_num") else sem.num


def _drop_own_engine_waits(nc, hoist_sem):
    """Remove same-engine semaphore waits from engine instructions.

    Tile sometimes gates an instruction on its own engine's completion
    semaphore (engine component runs behind the sequencer). Each engine
    executes and completes its instructions in order (PE matmuls are
    pc-monotone; DVE/ACT/Pool are strict FIFO), so these waits are
    redundant -- and walrus rejects instruction encodings with more than
    one sync wait (e.g. the matmul struct). InstLdweights is left alone:
    the PE may pull it ahead of in-flight matmuls.
    """
    own = {
        mybir.EngineType.PE: "PE_",
        mybir.EngineType.DVE: "DVE_",
        mybir.EngineType.Activation: "Activation_",
        mybir.EngineType.Pool: "Pool_",
    }
    for f in nc.m.functions:
        for blk in f.blocks:
            new_insts = []
            changed = False
            for inst in blk.instructions:
                si = getattr(inst, "sync_info", None)
                tn = type(inst).__name__
                if si is None or len(si.on_wait) <= 1:
                    new_insts.append(inst)
                    continue
                pre = own.get(inst.engine)
                if pre is not None and tn != "InstLdweights":
                    # own-engine waits are redundant for in-order engine ops
                    keep = [w for w in si.on_wait if not w.ant_name.startswith(pre)]
                else:
                    # Ldweights may be pulled ahead of in-flight matmuls, so
                    # keep its own-engine waits (hoisting to the sequencer
                    # preserves the gating); SP likewise keeps all waits.
                    keep = list(si.on_wait)
                # hoist all-but-one remaining wait onto engine NoOps that run
                # (in order) just before the instruction
                for w in keep[:-1]:
                    # carries one hoisted wait; updates a dedicated semaphore
                    # nothing waits on (sim requires every instruction to
                    # carry an update)
                    upd = mybir.SyncUpdate(
                        sync_type="semaphore",
                        id=w.id if False else _sem_id(nc, hoist_sem),
                        ant_name=hoist_sem.name,
                        update_mode="sem-inc",
                        update_value=1,
                        update_reg=None,
                    )
                    new_insts.append(
                        mybir.InstEventSemaphore(
                            name=f"{inst.name}-w{len(new_insts)}",
                            ins=[],
                            outs=[],
                            engine=inst.engine,
                            sync_info=mybir.SyncInfo(on_wait=[w], on_update=[upd]),
                        )
                    )
                inst.sync_info = mybir.SyncInfo(
                    on_wait=keep[-1:], on_update=si.on_update
                )
                new_insts.append(inst)
                changed = True
            if changed:
                blk.instructions[:] = new_insts


def _body(tc, T, TS, outT, hsT, oqT, wq, wk, wv, wo8, bq, bk, bo):
    nc = tc.nc
    NS = T // P      # number of 128-row s-tiles
    NT = T // TS     # number of t-blocks

    with (
        tc.tile_pool(name="cst", bufs=1) as cst,
        tc.tile_pool(name="sb", bufs=1) as sb,
        tc.tile_pool(name="work", bufs=3) as work,
        tc.tile_pool(name="ps", bufs=2, space="PSUM") as ps,
    ):
        # ---- constants -------------------------------------------------
        ones32 = cst.tile([1, DH], BF16, tag="ones32")
        nc.vector.memset(ones32[:], 1.0)
        w_sb = {}
        for name, w in (("wq", wq), ("wk", wk), ("wv", wv)):
            tls = []
            for i in range(2):
                t_ = cst.tile([P, E], BF16, tag=f"{name}{i}", name=f"{name}_{i}")
                nc.sync.dma_start(t_[:], w[ts(i, P), :])
                tls.append(t_)
            w_sb[name] = tls
        wo8_sb = cst.tile([DH, H, E], BF16, tag="wo8")
        nc.sync.dma_start(wo8_sb[:], wo8.rearrange("d (h e) -> d h e", h=H))
        b_sb = {}
        for name, b in (("bq", bq), ("bk", bk), ("bo", bo)):
            tls = []
            for i in range(2):
                t_ = cst.tile([P, 1], F32, tag=f"{name}{i}", name=f"{name}_{i}")
                nc.sync.dma_start(t_[:], b[ts(i, P), :])
                # route through a DVE copy so downstream users depend on DVE
                # (same engine as the evacuation ops) instead of the DMA --
                # keeps every compute instruction at <=1 cross-engine wait
                # (walrus rejects multi-wait matmul/TT encodings).
                t2_ = cst.tile([P, 1], F32, tag=f"{name}c{i}", name=f"{name}c_{i}")
                nc.vector.tensor_copy(t2_[:], t_[:])
                tls.append(t2_)
            b_sb[name] = tls

        # ---- load activations ------------------------------------------
        hs, oq = [], []
        for i in range(2):
            t_ = sb.tile([P, T], F32, tag=f"hs{i}", name=f"hs_{i}")
            nc.sync.dma_start(t_[:], hsT[ts(i, P), :])
            hs.append(t_)
            t_ = sb.tile([P, T], F32, tag=f"oq{i}", name=f"oq_{i}")
            nc.sync.dma_start(t_[:], oqT[ts(i, P), :])
            oq.append(t_)
        hsp, hid = [], []
        for i in range(2):
            a = sb.tile([P, T], BF16, tag=f"hsp{i}", name=f"hsp_{i}")
            nc.vector.tensor_add(a[:], hs[i][:], oq[i][:])
            hsp.append(a)
            c = sb.tile([P, T], BF16, tag=f"hid{i}", name=f"hid_{i}")
            nc.vector.tensor_copy(c[:], hs[i][:])
            hid.append(c)

        # ---- q/k projections: out qT/kT [E, T] bf16 --------------------
        def proj_qk(wname, bias_tiles, out_tag):
            outs = []
            for m in range(2):
                o = sb.tile([P, T], BF16, tag=f"{out_tag}{m}", name=f"{out_tag}_{m}")
                for c2 in range(T // TS):
                    pt = ps.tile([P, TS], F32, tag="scores", name=f"pp_{out_tag}{m}_{c2}")
                    for k in range(2):
                        nc.tensor.matmul(
                            pt[:],
                            w_sb[wname][k][:, ts(m, P)],
                            hsp[k][:, ts(c2, TS)],
                            start=(k == 0),
                            stop=(k == 1),
                        )
                    nc.vector.tensor_scalar_add(
                        o[:, ts(c2, TS)], pt[:], bias_tiles[m]
                    )
                outs.append(o)
            return outs

        qt = proj_qk("wq", b_sb["bq"], "qt")
        kt = proj_qk("wk", b_sb["bk"], "kt")

        # ---- v' projection: vprime[p, s_tile, h, 0:32]=v, [...,32]=1 ---
        vprime = sb.tile([P, NS, H, DH + 1], BF16, tag="vprime")
        nc.vector.memset(vprime[:, :, :, DH: DH + 1], 1.0)
        for st in range(NS):
            pv = ps.tile([P, E], F32, tag="num", bufs=4, name=f"pv_{st}")
            for k in range(2):
                nc.tensor.matmul(
                    pv[:],
                    hid[k][:, ts(st, P)],
                    w_sb["wv"][k][:],
                    start=(k == 0),
                    stop=(k == 1),
                )
            nc.vector.tensor_copy(
                vprime[:, st, :, 0:DH],
                pv[:].rearrange("p (h d) -> p h d", h=H),
            )

        # ---- attention -------------------------------------------------
        # Software-pipelined over head-subgroups: the PV accumulation chains
        # of subgroup j run interleaved with the QK+exp s-loop of subgroup
        # j+1 (carried across t-blocks), so the ScalarE exp stream never
        # waits on PE-side PV/projection work.
        attn_p = {}   # (tsup, h) -> attn piece [32, TS] bf16 (partitions 0-31)

        def emit_pv_step(prev, s):
            for hh in range(2):
                h = 2 * prev["g2"] + hh
                nc.tensor.matmul(
                    prev["nm"][hh][0: DH + 1, :],
                    vprime[:, s, h, :],
                    prev["exs"][s][:, ts(hh, TS)],
                    start=(s == 0),
                    stop=(s == NS - 1),
                )

        def emit_outproj(tsup):
            tsl = ts(tsup, TS)
            for m in range(2):
                op = ps.tile([P, TS], F32, tag="num", bufs=4,
                             name=f"op{tsup}_{m}")
                for h in range(H):
                    nc.tensor.matmul(
                        op[:],
                        wo8_sb[:, h, ts(m, P)],
                        attn_p[(tsup, h)][:],
                        start=(h == 0),
                        stop=(h == H - 1),
                    )
                ob = work.tile([P, TS], F32, tag="osb", name=f"ob{tsup}_{m}")
                nc.vector.tensor_scalar_add(ob[:], op[:], b_sb["bo"][m])
                nc.sync.dma_start(outT[ts(m, P), tsl], ob[:])

        def finish_prev(prev):
            tsup = prev["tsup"]
            for hh in range(2):
                h = 2 * prev["g2"] + hh
                nm = prev["nm"][hh]
                # normalize: attn = num[0:32] / num[32]
                r_ = work.tile([P, TS], BF16, tag="recip", bufs=4,
                               name=f"r{tsup}_{h}")
                with nc.allow_low_precision(
                    reason="recip(den) in bf16: uniform per-column scale, "
                           "well inside tolerance"
                ):
                    nc.vector.reciprocal(r_[DH: DH + 1, :], nm[DH: DH + 1, :])
                # bounce recip(den) to partition 0, then PE-broadcast it
                # across partitions 0..31 with a K=1 ones matmul
                r0 = work.tile([1, TS], BF16, tag="r0", bufs=4,
                               name=f"r0_{tsup}_{h}")
                nc.sync.dma_start(r0[0:1, :], r_[DH: DH + 1, :])
                rbp = ps.tile([DH, TS], F32, tag="num", bufs=4,
                              name=f"rb{tsup}_{h}")
                nc.tensor.matmul(
                    rbp[:], ones32[:], r0[0:1, :], start=True, stop=True
                )
                rbs = work.tile([DH, TS], F32, tag="rbs", bufs=4,
                                name=f"rbs{tsup}_{h}")
                nc.vector.tensor_copy(rbs[:], rbp[:])
                ap_ = work.tile([DH, TS], BF16, tag="attnp", bufs=2 * H + 2,
                                name=f"ap{tsup}_{h}")
                nc.vector.tensor_mul(ap_[:], nm[0: DH, :], rbs[:])
                attn_p[(tsup, h)] = ap_
            if prev["g2"] == 3:
                emit_outproj(tsup)

        prev = None
        for tsup in range(NT):
            tsl = ts(tsup, TS)
            for g2 in range(4):          # head subgroups (2*g2, 2*g2+1)
                exs = []
                for s in range(NS):
                    sc = ps.tile([P, 2 * TS], F32, tag="scores",
                                 name=f"sc{tsup}_{g2}_{s}")
                    for hh in range(2):
                        h = 2 * g2 + hh
                        r = h % 4
                        nc.tensor.matmul(
                            sc[:, ts(hh, TS)],
                            kt[h // 4][32 * r: 32 * r + 32, ts(s, P)],
                            qt[h // 4][32 * r: 32 * r + 32, tsl],
                            start=True,
                            stop=True,
                            tile_position=(32 * r, 0),
                        )
                    ex = work.tile([P, 2 * TS], BF16, tag="expT",
                                   bufs=2 * NS + 4,
                                   name=f"ex{tsup}_{g2}_{s}")
                    nc.scalar.activation(ex[:], sc[:], AF.Exp, scale=SCALING)
                    exs.append(ex)
                    if prev is not None:
                        emit_pv_step(prev, s)
                if prev is not None:
                    finish_prev(prev)
                prev = {
                    "tsup": tsup,
                    "g2": g2,
                    "exs": exs,
                    "nm": [
                        ps.tile([P, TS], F32, tag="num", bufs=4,
                                name=f"num{tsup}_{2 * g2 + hh}")
                        for hh in range(2)
                    ],
                }
        # drain the last subgroup
        for s in range(NS):
            emit_pv_step(prev, s)
        finish_prev(prev)


# ----------------------------------------------------------------------
# host-side wrapper
# ----------------------------------------------------------------------

_BUILT = {}


def _get_nc(T):
    if T not in _BUILT:
        _BUILT[T] = build_nc(T)
    return _BUILT[T]


def prep_weights(Wq, bq, Wk, bk, Wv, bv, Wo, bo):
    """Shared (batch-independent) input arrays."""
    bf = ml_dtypes.bfloat16
    f32 = np.float32

    def wt(w):
        return np.ascontiguousarray(np.asarray(w, f32).T).astype(bf)

    woT = np.asarray(Wo, f32).T            # [d_in=256, e_out=256]
    wo8 = np.ascontiguousarray(
        woT.reshape(H, DH, E).transpose(1, 0, 2).reshape(DH, H * E)
    ).astype(bf)
    # softmax rows sum to 1, so the value bias passes straight through
    # attention: out = (num0/den) @ Wo.T + (bo + Wo @ bv)
    bo_eff = np.asarray(bo, f32) + np.asarray(Wo, f32) @ np.asarray(bv, f32)
    return {
        "wq": wt(Wq),
        "wk": wt(Wk),
        "wv": wt(Wv),
        "wo8": wo8,
        "bq": np.asarray(bq, f32).reshape(E, 1).copy(),
        "bk": np.asarray(bk, f32).reshape(E, 1).copy(),
        "bo": bo_eff.reshape(E, 1).copy(),
    }


def prep_core_inputs(hidden_b, obj_b, Wq, bq, Wk, bk, Wv, bv, Wo, bo):
    """Per-core input dict for one batch element. hidden_b/obj_b: [T, E] f32."""
    d = prep_weights(Wq, bq, Wk, bk, Wv, bv, Wo, bo)
    d["hsT"] = np.ascontiguousarray(np.asarray(hidden_b, np.float32).T)
    d["oqT"] = np.ascontiguousarray(np.asarray(obj_b, np.float32).T)
    return d


def _numpy_reference(hidden, obj, mask, Wq, bq, Wk, bk, Wv, bv, Wo, bo):
    """Exact fp32 fallback (only used if the mask is ever nonzero)."""
    hs_pos = hidden + obj
    q = (hs_pos @ Wq.T + bq) * SCALING
    k = hs_pos @ Wk.T + bk
    v = hidden @ Wv.T + bv
    b, t, _ = hidden.shape

    def split(x):
        return x.reshape(b, t, H, DH).transpose(0, 2, 1, 3)

    q, k, v = split(q), split(k), split(v)
    out = np.empty((b, H, t, DH), np.float32)
    for bi in range(b):
        for hi in range(H):
            s = q[bi, hi] @ k[bi, hi].T + mask[bi, 0]
            s = s - s.max(axis=-1, keepdims=True)
            e = np.exp(s)
            p = e / e.sum(axis=-1, keepdims=True)
            out[bi, hi] = p @ v[bi, hi]
    out = out.transpose(0, 2, 1, 3).reshape(hidden.shape)
    return out @ Wo.T + bo


def kernel(hidden_states, object_queries, attention_mask,
           Wq, bq, Wk, bk, Wv, bv, Wo, bo):
    hidden = np.asarray(hidden_states, np.float32)
    obj = np.asarray(object_queries, np.float32)
    mask = np.asarray(attention_mask, np.float32)
    b, t, _ = hidden.shape
    assert b == B and hidden.shape[2] == E

    if mask.any():
        return _numpy_reference(
            hidden, obj, mask,
            np.asarray(Wq, np.float32), np.asarray(bq, np.float32),
            np.asarray(Wk, np.float32), np.asarray(bk, np.float32),
            np.asarray(Wv, np.float32), np.asarray(bv, np.float32),
            np.asarray(Wo, np.float32), np.asarray(bo, np.float32),
        ).astype(np.float32)

    nc = _get_nc(t)
    shared = prep_weights(Wq, bq, Wk, bk, Wv, bv, Wo, bo)
    in_maps = []
    for i in range(B):
        d = dict(shared)
        d["hsT"] = np.ascontiguousarray(hidden[i].T)
        d["oqT"] = np.ascontiguousarray(obj[i].T)
        in_maps.append(d)
    res = bass_utils.run_bass_kernel_spmd(nc, in_maps, core_ids=list(range(NCORES)))
    out = np.stack([res.results[i]["outT"].T for i in range(B)])
    return np.ascontiguousarray(out.astype(np.float32))

